# revision 61
# baseline (speedup 1.0000x reference)
"""Trainium2 Bass kernel v3: dual-softmax ("contrast") multi-head self-attention.

Problem (per full input):
  x, y: (4, 1024, 1024) f32; Wq/Wk/Wv: (1024, 1024) f32, nh=16 heads, dk=dv=64.
  dist   = softmax(q k^T / 8)
  c_att  = softmax(1 - dist) @ v      (== softmax(-dist) @ v)
  att    = softmax(dist) @ v
Sharding: 8 cores = 4 batches x 2 head-groups (8 heads each).

v3 design notes (changes vs v2.2, driven by the p-state clock model):
  * The PE runs at 2.4 GHz only after ~3us of CONTINUOUS busy; fragmented
    bursts run at 1.2 GHz. v2.2's attention phase was PE-paced at the mid
    p-state (~26us/head). v3 packs PE work into large bursts (all 128 O
    matmuls of a head emitted as one early-iteration block, kb-outer so
    the late e3/e2 tiles are needed last) and sizes everything so the ACT
    engine (e1+e3 exps, ~17us/head -- the irreducible floor) sets the pace.
  * Broadcast-Z1: the Z1 row-sum matmuls use an ALL-ONES [128,128]
    stationary, so Z1 lands replicated across all 128 PSUM partitions for
    the same streaming cost as a 1-row ones matmul. r1full = approx-recip
    straight from that PSUM to bf16 SBUF -- two DVE ops replace v2.2's
    [1,N] recips + GpSimd partition_broadcast. Critically this removes the
    GpSimd extended-instruction library entirely: alternating tensor ops
    and partition_broadcast on the Pool forces a ~5us library reload DMA
    on every switch, which sat on the cross-iteration critical chain.
  * O matmuls grouped 3 query-ptiles per PSUM bank (130 f32 cols each):
    ONE start=True zeroes the bank, everything accumulates (relies on the
    HW zeroing the full 2KB region, as v2.2's 2-branch trick already did).
    2 group tiles rotate; o_norm drains a group with per-column fast
    recips + 6 tensor_scalar_muls.
  * PSUM budget (8 banks): stp 2x[128,1024]f32 = 4, z1 2x[128,512]f32 = 2,
    opp 2x[128,512]f32 = 2.
  * dist multiplies on the Pool except kb=7 (DVE): the Pool is idle at the
    iteration boundary where dist(b,0) must fire instantly; measured Pool
    tensor_mul is ~2.1us vs DVE 0.6-1.7us ([128,1024] bf16 -- both degrade
    under concurrent SBUF traffic), and 7-on-Pool measured fastest.
  * phase-1 tail (Z1 finish + r1full recips) emitted at the END of the
    producing iteration, before the e2 leftovers in the DVE queue, so the
    next head's dist chain starts with zero added latency.
  * V projection + y/wv transposes overlap head-0 phase 1 (ST/e1/Z1);
    V PSUM rides the stp ring; e3/e2 pools open only after the setup pool
    closes (SBUF overlay). Outputs DMA per qb at the last head's o_norm.
"""

import sys

if "/opt/trn_rl_repo" not in sys.path:
    sys.path.insert(0, "/opt/trn_rl_repo")

from contextlib import ExitStack

import numpy as np

import concourse.bass as bass
from concourse import bacc, masks, mybir
from concourse.bass_utils import run_bass_kernel_spmd
from concourse.dve_ops import RECIP_APPROX_FAST_CONSTS, RECIPROCAL_APPROX_FAST
from concourse.tile import TileContext

F32 = mybir.dt.float32
BF16 = mybir.dt.bfloat16
F8 = mybir.dt.float8e4
EXP = mybir.ActivationFunctionType.Exp

P = 128          # partitions
N = 1024         # tokens
D = 1024         # model dim
NF = 512         # features per core (8 heads x 64)
FH = 8           # heads per core
DK = 64          # head dim
NPT = N // P     # 8 token ptiles
KBN = D // P     # 8 contraction blocks
MB = NF // P     # 4 feature ptiles

# O-matmul qb grouping: 3 groups of query-ptiles per head sharing PSUM banks
O_GROUPS = ((0, 1, 2), (3, 4, 5), (6, 7))
GW = 2 * DK + 2          # 130 f32 cols per qb in a group tile
DIST_DVE_KBS = (7,)       # dist blocks routed to DVE (rest on Pool). The
                          # boundary-critical dist(b, 0) runs on the Pool,
                          # which is idle there.


def _recip_fast(nc, out, in0):
    """~51-ULP approximate reciprocal, 1 DVE instruction."""
    cc = RECIP_APPROX_FAST_CONSTS
    nc.vector._custom_dve(
        RECIPROCAL_APPROX_FAST, out=out, in0=in0,
        s0=cc["s0"], s1=cc["s1"], imm2=cc["imm2"],
    )


def build_nc():
    nc = bacc.Bacc("TRN2")
    x_d = nc.dram_tensor("x", [N, D], F32, kind="ExternalInput")
    y_d = nc.dram_tensor("y", [N, D], F32, kind="ExternalInput")
    wq_d = nc.dram_tensor("wq", [NF, D], F32, kind="ExternalInput")
    wk_d = nc.dram_tensor("wk", [NF, D], F32, kind="ExternalInput")
    wv_d = nc.dram_tensor("wv", [NF, D], F32, kind="ExternalInput")
    catt_d = nc.dram_tensor("catt", [N, NF], F32, kind="ExternalOutput")
    att_d = nc.dram_tensor("att", [N, NF], F32, kind="ExternalOutput")

    with TileContext(nc) as tc, ExitStack() as ctx:
        persist = ctx.enter_context(tc.tile_pool(name="persist", bufs=1))
        ident = persist.tile([P, P], F32)
        masks.make_identity(nc, ident[:])
        ident_b = persist.tile([P, P], BF16)
        nc.vector.tensor_copy(ident_b[:], ident[:])

        qt = persist.tile([P, MB, N], BF16)       # Q^T: [feat%128, featblk, tok]
        kt = persist.tile([P, MB, N], BF16)
        vv = persist.tile([P, NPT, FH, DK + 1], BF16)  # V_aug per head
        att_sb = persist.tile([P, NPT, NF], F32)
        catt_sb = persist.tile([P, NPT, NF], F32)
        ones_mat = persist.tile([P, P], BF16)
        nc.vector.memset(ones_mat[:], 1.0)
        nc.vector.memset(vv[:, :, :, DK:DK + 1], 1.0)

        # ---------------- setup part 1: x, Wq, Wk, Q/K projections ---------
        xt = persist.tile([P, KBN, N], BF16)
        wqt = persist.tile([P, KBN, 512], BF16)
        wkt = persist.tile([P, KBN, 512], BF16)

        def eng_copy(idx, dst, src):
            if idx % 2 == 0:
                nc.vector.tensor_copy(dst, src)
            else:
                nc.scalar.copy(dst, src)

        with ExitStack() as s1ctx:
            s1p = s1ctx.enter_context(tc.tile_pool(name="s1p", bufs=1))
            pst1 = s1ctx.enter_context(
                tc.tile_pool(name="pst1", bufs=4, space="PSUM"))
            prj = s1ctx.enter_context(
                tc.tile_pool(name="prj", bufs=2, space="PSUM"))

            xb = s1p.tile([P, NPT, D], BF16, tag="xb")
            for i in range(NPT):
                raw = s1p.tile([P, D], F32, tag="rx", bufs=2, name="raw")
                nc.sync.dma_start(out=raw[:], in_=x_d[i * P:(i + 1) * P, :])
                nc.scalar.copy(xb[:, i, :], raw[:])

            # w casts all on DVE -- they run in parallel with the x casts
            # on ACT instead of behind them
            def load_wt(w_d, wt):
                wb = s1p.tile([P, MB, D], BF16, tag="wb", bufs=2, name="wb")
                for m in range(MB):
                    wraw = s1p.tile([P, D], F32, tag="wr", bufs=2, name="wraw")
                    nc.sync.dma_start(out=wraw[:], in_=w_d[m * P:(m + 1) * P, :])
                    nc.vector.tensor_copy(wb[:, m, :], wraw[:])
                for kb in range(KBN):
                    tp = pst1.tile([P, 512], BF16, tag="tp")
                    for m in range(MB):
                        nc.tensor.transpose(
                            tp[:, m * P:(m + 1) * P],
                            wb[:, m, kb * P:(kb + 1) * P],
                            ident_b[:],
                        )
                    eng_copy(kb, wt[:, kb, :], tp[:])

            cidx = 0
            for half in range(2):
                for kb in range(KBN):
                    tp = pst1.tile([P, 512], BF16, tag="tp")
                    for j in range(4):
                        i = half * 4 + j
                        nc.tensor.transpose(
                            tp[:, j * P:(j + 1) * P],
                            xb[:, i, kb * P:(kb + 1) * P],
                            ident_b[:],
                        )
                    eng_copy(cidx, xt[:, kb, half * 512:(half + 1) * 512],
                             tp[:])
                    cidx += 1

            load_wt(wq_d, wqt)
            load_wt(wk_d, wkt)
            for w_sb, out_sb in ((wqt, qt), (wkt, kt)):
                for m in range(MB):
                    q_ps = prj.tile([P, N], F32, tag="proj")
                    for ch in range(2):
                        for kb in range(KBN):
                            nc.tensor.matmul(
                                q_ps[:, ch * 512:(ch + 1) * 512],
                                lhsT=w_sb[:, kb, m * P:(m + 1) * P],
                                rhs=xt[:, kb, ch * 512:(ch + 1) * 512],
                                start=(kb == 0),
                                stop=(kb == KBN - 1),
                            )
                    eng_copy(m, out_sb[:, m, :], q_ps[:])

        # attention PSUM pools (opened after setup part 1 frees its banks)
        stp = ctx.enter_context(tc.tile_pool(name="stp", bufs=2, space="PSUM"))
        z1p = ctx.enter_context(tc.tile_pool(name="z1p", bufs=1, space="PSUM"))

        # ---------------- attention machinery ------------------------------
        e1p = ctx.enter_context(tc.tile_pool(name="e1p", bufs=17))
        r1f = ctx.enter_context(tc.tile_pool(name="r1f", bufs=2))
        smp = ctx.enter_context(tc.tile_pool(name="smp", bufs=6))

        e1_tiles = {}
        e3_tiles = {}
        e2_tiles = {}
        z1_tiles = {}
        r1f_tiles = {}
        o_tiles = {}

        # e3/e2 pools are opened AFTER the setup pool closes (SBUF overlay);
        # placeholder set later.
        pools = {}

        def st_e1(h, kb):
            """S^T block matmuls + e1t = exp(ST/8) (single ACT op / kb)."""
            hb, ho = h // 2, (h % 2) * DK
            st = stp.tile([P, N], F32, tag="st")
            for ch in range(2):
                nc.tensor.matmul(
                    st[:, ch * 512:(ch + 1) * 512],
                    lhsT=kt[ho:ho + DK, hb, kb * P:(kb + 1) * P],
                    rhs=qt[ho:ho + DK, hb, ch * 512:(ch + 1) * 512],
                    start=True,
                    stop=True,
                )
            e1 = e1p.tile([P, N], BF16, tag="e1")
            nc.scalar.activation(e1[:], st[:], EXP, scale=0.125)
            e1_tiles[h].append(e1)

        def z1_mms(h, kb):
            """Z1 accumulation with an ALL-ONES stationary: out[i, q] =
            sum_k e1[k, q] for every i -- the row sum lands pre-broadcast
            across all 128 partitions, for the same streaming cost as a
            1-row ones matmul."""
            for ch, z1t in ((0, z1_tiles[h][0]), (1, z1_tiles[h][1])):
                nc.tensor.matmul(
                    z1t[:, :],
                    lhsT=ones_mat[:, :],
                    rhs=e1_tiles[h][kb][:, ch * 512:(ch + 1) * 512],
                    start=(kb == 0),
                    stop=(kb == KBN - 1),
                )

        def z1_alloc(h):
            z1_tiles[h] = (
                z1p.tile([P, 512], F32, tag="z1a", name="z1a"),
                z1p.tile([P, 512], F32, tag="z1b", name="z1b"),
            )

        def phase1_tail(h):
            """r1full = approx 1/Z1, straight from the broadcast Z1 PSUM to
            bf16 SBUF -- two DVE ops, no partition broadcast, no copies."""
            z1a, z1b = z1_tiles.pop(h)
            r1full = r1f.tile([P, N], BF16, tag="r1f")
            _recip_fast(nc, r1full[:, 0:512], z1a[:, :])
            _recip_fast(nc, r1full[:, 512:N], z1b[:, :])
            r1f_tiles[h] = r1full

        def dist(h, kb):
            """dist^T = e1t * r1 (in place, bf16), split Pool/DVE."""
            e1 = e1_tiles[h][kb]
            eng = nc.vector if kb in DIST_DVE_KBS else nc.gpsimd
            eng.tensor_mul(e1[:], e1[:], r1f_tiles[h][:])

        def e3_op(h, kb):
            e3 = pools["e3p"].tile([P, N], BF16, tag="e3")
            nc.scalar.activation(e3[:], e1_tiles[h][kb][:], EXP)
            e3_tiles[h].append(e3)

        def e2_op(h, kb):
            e2 = pools["e2p"].tile([P, N], BF16, tag="e2")
            _recip_fast(nc, e2[:], e3_tiles[h][kb][:])
            e2_tiles[h].append(e2)

        def o_group(h, g):
            """All O matmuls for qb group g of head h into ONE PSUM bank.
            First matmul start=True zeroes the whole 2KB bank; all others
            accumulate. Layout per qb j: cols [j*GW, j*GW+65) = att branch,
            [j*GW+65, j*GW+130) = catt branch. kb-outer order: the late
            e3/e2(h, 6..7) tiles (finishing early this iteration) are only
            needed near the end of the burst, so the PE never stalls."""
            qbs = O_GROUPS[g]
            o_ps = pools["opp"].tile([P, 512], F32, tag="o")
            first = True
            for kb in range(KBN):
                for j, qb in enumerate(qbs):
                    base = j * GW
                    nc.tensor.matmul(
                        o_ps[:, base:base + DK + 1],
                        lhsT=e3_tiles[h][kb][:, qb * P:(qb + 1) * P],
                        rhs=vv[:, kb, h, :],
                        start=first,
                        stop=False,
                        skip_group_check=True,
                    )
                    first = False
                    nc.tensor.matmul(
                        o_ps[:, base + DK + 1:base + 2 * (DK + 1)],
                        lhsT=e2_tiles[h][kb][:, qb * P:(qb + 1) * P],
                        rhs=vv[:, kb, h, :],
                        start=False,
                        stop=(kb == KBN - 1),
                        skip_group_check=True,
                    )
            o_tiles[(h, g)] = o_ps

        def o_norm_group(h, g):
            """Normalize + store + DMA one qb group. One strided approx-recip
            covers all denominators (cols j*GW+64 and j*GW+129, stride 65)."""
            qbs = O_GROUPS[g]
            o_ps = o_tiles.pop((h, g))
            rr = smp.tile([P, 2 * len(O_GROUPS[0])], F32, tag="rr")
            for j in range(len(qbs)):
                base = j * GW
                _recip_fast(nc, rr[:, 2 * j:2 * j + 1],
                            o_ps[:, base + DK:base + DK + 1])
                _recip_fast(nc, rr[:, 2 * j + 1:2 * j + 2],
                            o_ps[:, base + 2 * DK + 1:base + 2 * DK + 2])
            for j, qb in enumerate(qbs):
                base = j * GW
                hc = slice(h * DK, (h + 1) * DK)
                nc.vector.tensor_scalar_mul(
                    att_sb[:, qb, hc], o_ps[:, base:base + DK],
                    rr[:, 2 * j:2 * j + 1],
                )
                nc.vector.tensor_scalar_mul(
                    catt_sb[:, qb, hc], o_ps[:, base + DK + 1:base + 2 * DK + 1],
                    rr[:, 2 * j + 1:2 * j + 2],
                )
                # per-(head, qb) output slices: spreads the 4MB of output
                # DMA across the attention phase instead of an ~11us tail
                nc.sync.dma_start(
                    out=att_d[qb * P:(qb + 1) * P, h * DK:(h + 1) * DK],
                    in_=att_sb[:, qb, hc])
                nc.sync.dma_start(
                    out=catt_d[qb * P:(qb + 1) * P, h * DK:(h + 1) * DK],
                    in_=catt_sb[:, qb, hc])

        # ---------------- setup part 2: head-0 phase-1 + V pipeline --------
        # PSUM here: stp(4) + z1(2) + pst2(2) = 8 banks.
        sctx = ExitStack()
        sbp = sctx.enter_context(tc.tile_pool(name="setup", bufs=1))
        with ExitStack() as s2ctx:
            pst2 = s2ctx.enter_context(
                tc.tile_pool(name="pst2", bufs=1, space="PSUM"))

            yb = sbp.tile([P, NPT, D], BF16, tag="yb")
            yt = sbp.tile([P, KBN, N], BF16, tag="yt")
            # y DMA + casts first in the DVE queue
            for i in range(NPT):
                yraw = sbp.tile([P, D], F32, tag="ry", bufs=2, name="yraw")
                nc.sync.dma_start(out=yraw[:], in_=y_d[i * P:(i + 1) * P, :])
                nc.vector.tensor_copy(yb[:, i, :], yraw[:])

            e1_tiles[0] = []
            st_e1(0, 0)
            st_e1(0, 1)

            def ytp_bundle(kb, half, cidx):
                tp = pst2.tile([P, 512], BF16, tag="tp", bufs=2, name="tp")
                for j in range(4):
                    i = half * 4 + j
                    nc.tensor.transpose(
                        tp[:, j * P:(j + 1) * P],
                        yb[:, i, kb * P:(kb + 1) * P],
                        ident_b[:],
                    )
                eng_copy(cidx, yt[:, kb, half * 512:(half + 1) * 512], tp[:])

            # interleave: y transposes (half 0) + head-0 ST/e1/Z1
            s = 2
            for kb in range(KBN):
                ytp_bundle(kb, 0, kb)
                if kb % 2 == 1 and s < KBN:
                    st_e1(0, s)
                    if s == 2:
                        z1_alloc(0)
                    z1_mms(0, s - 2)
                    s += 1

            # Wv load (casts on DVE after y casts)
            wvb = sbp.tile([P, MB, D], BF16, tag="wvb")
            for m in range(MB):
                wraw = sbp.tile([P, D], F32, tag="wvr", bufs=2, name="wvraw")
                nc.sync.dma_start(out=wraw[:], in_=wv_d[m * P:(m + 1) * P, :])
                nc.vector.tensor_copy(wvb[:, m, :], wraw[:])
            wvt = sbp.tile([P, KBN, 512], BF16, tag="wvt")

            for kb in range(KBN):
                ytp_bundle(kb, 1, kb)
                if kb % 2 == 1 and s < KBN:
                    st_e1(0, s)
                    z1_mms(0, s - 2)
                    s += 1

            for kb in range(KBN):
                tp = pst2.tile([P, 512], BF16, tag="tp", bufs=2, name="tp")
                for m in range(MB):
                    nc.tensor.transpose(
                        tp[:, m * P:(m + 1) * P],
                        wvb[:, m, kb * P:(kb + 1) * P],
                        ident_b[:],
                    )
                eng_copy(kb, wvt[:, kb, :], tp[:])

            # V projection; PSUM rides the stp ring (head-0 STs are done by
            # now, so the ring slots are free); copies alternate ACT/DVE
            for i in range(NPT):
                v_ps = stp.tile([P, N], F32, tag="st", name="vps")
                for kb in range(KBN):
                    nc.tensor.matmul(
                        v_ps[:, 0:512],
                        lhsT=yt[:, kb, i * P:(i + 1) * P],
                        rhs=wvt[:, kb, :],
                        start=(kb == 0),
                        stop=(kb == KBN - 1),
                    )
                if i % 2 == 0:
                    nc.scalar.copy(
                        vv[:, i, :, 0:DK],
                        v_ps[:, 0:512].rearrange("p (h d) -> p h d", h=FH),
                    )
                else:
                    nc.vector.tensor_copy(
                        vv[:, i, :, 0:DK],
                        v_ps[:, 0:512].rearrange("p (h d) -> p h d", h=FH),
                    )
            # finish head-0 Z1 + its phase-1 tail (r1/bcast before iter 1)
            z1_mms(0, 6)
            z1_mms(0, KBN - 1)
            phase1_tail(0)

        sctx.close()  # free setup SBUF; open e3/e2/opp pools in its place
        pools["e3p"] = ctx.enter_context(tc.tile_pool(name="e3p", bufs=18))
        pools["e2p"] = ctx.enter_context(tc.tile_pool(name="e2p", bufs=18))
        pools["opp"] = ctx.enter_context(
            tc.tile_pool(name="opp", bufs=2, space="PSUM"))

        def proj_half(w_sb, dst_sb, m, ch):
            """Deferred Q/K projection (feature block m, token half ch),
            riding the opp PSUM ring during the attention iterations."""
            t = pools["opp"].tile([P, 512], F32, tag="o", name="pj")
            for kb in range(KBN):
                nc.tensor.matmul(
                    t[:, :],
                    lhsT=w_sb[:, kb, m * P:(m + 1) * P],
                    rhs=xt[:, kb, ch * 512:(ch + 1) * 512],
                    start=(kb == 0),
                    stop=(kb == KBN - 1),
                )
            nc.vector.tensor_copy(dst_sb[:, m, ch * 512:(ch + 1) * 512],
                                  t[:, :])

        # deferred-projection schedule -- empty: the S2/iteration-1 overlap
        # it created cost more (SBUF contention stalling the V pipeline)
        # than the shorter serial setup saved
        PROJ_END = {}

        # ---------------- attention: pipelined head loop -------------------
        # iter it: c = it (ST/e1/Z1), b = it-1 (r1/dist/e3/e2), a = it-2 (O)
        for it in range(1, FH + 2):
            c = it if it < FH else None
            b = it - 1 if it - 1 < FH else None
            a = it - 2

            if c is not None:
                e1_tiles[c] = []
            if b is not None:
                e3_tiles[b] = []
                e2_tiles[b] = []

            # r1full(b) was computed at the end of the previous iteration,
            # so dist(b, 0) can fire immediately. The ACT queue leads with
            # THREE e1 ops before the first e3 so the stp ring drains ahead
            # of the JIT ST feed.
            if c is not None:
                st_e1(c, 0)
            if b is not None:
                dist(b, 0)
            if c is not None:
                st_e1(c, 1)

            # all three O groups as ONE early PE burst (long enough to ramp
            # the PE p-state); norms emitted right after so the DVE frees
            # the 2-bank ring before group 2 needs its slot back.
            if a >= 0:
                o_group(a, 0)
                o_norm_group(a, 0)
                o_group(a, 1)
                o_norm_group(a, 1)
                o_group(a, 2)
                o_norm_group(a, 2)

            # ST/e1 feed, just-in-time for the ACT queue (e3 trails e1 by 2)
            for s in range(2, KBN):
                if c is not None:
                    st_e1(c, s)
                    if s == 2:
                        z1_alloc(c)
                    z1_mms(c, s - 2)
                if b is not None:
                    dist(b, s - 1)
                    e3_op(b, s - 2)
                    if s >= 3:
                        e2_op(b, s - 3)


            # leftovers; phase-1 recips BEFORE the last e2s in the DVE queue
            # (the next iteration's dist chain needs r1full early; the e2s
            # are only consumed by the O burst one iteration later)
            if c is not None:
                z1_mms(c, 6)
            if b is not None:
                dist(b, 7)
                e3_op(b, 6)
                e3_op(b, 7)
            if c is not None:
                z1_mms(c, KBN - 1)
                phase1_tail(c)
            if b is not None:
                e2_op(b, 5)
                e2_op(b, 6)
                e2_op(b, 7)
                del e1_tiles[b]
            for w_sb, dst, m, ch in PROJ_END.get(it, ()):
                proj_half(w_sb, dst, m, ch)
            if a >= 0:
                del e3_tiles[a], e2_tiles[a]

    nc.finalize()
    return nc


_NC_CACHE = {}


def _get_nc():
    if "nc" not in _NC_CACHE:
        _NC_CACHE["nc"] = build_nc()
    return _NC_CACHE["nc"]


def _make_in_maps(x, y, Wq, Wk, Wv):
    x = np.ascontiguousarray(np.asarray(x, dtype=np.float32))
    y = np.ascontiguousarray(np.asarray(y, dtype=np.float32))
    Wq = np.ascontiguousarray(np.asarray(Wq, dtype=np.float32))
    Wk = np.ascontiguousarray(np.asarray(Wk, dtype=np.float32))
    Wv = np.ascontiguousarray(np.asarray(Wv, dtype=np.float32))
    in_maps = []
    for c in range(8):
        b, h0 = c // 2, (c % 2) * 8
        rows = slice(h0 * DK, h0 * DK + NF)
        in_maps.append({
            "x": x[b],
            "y": y[b],
            "wq": np.ascontiguousarray(Wq[rows]),
            "wk": np.ascontiguousarray(Wk[rows]),
            "wv": np.ascontiguousarray(Wv[rows]),
        })
    return in_maps


def run_cores(x, y, Wq, Wk, Wv, trace=False, tmpdir=None):
    nc = _get_nc()
    res = run_bass_kernel_spmd(
        nc, _make_in_maps(x, y, Wq, Wk, Wv), core_ids=list(range(8)),
        trace=trace, tmpdir=tmpdir,
    )
    B = 4
    c_att = np.empty((B, N, 2 * NF), dtype=np.float32)
    att = np.empty((B, N, 2 * NF), dtype=np.float32)
    for c, r in enumerate(res.results):
        b, cols = c // 2, slice((c % 2) * NF, (c % 2) * NF + NF)
        c_att[b][:, cols] = r["catt"]
        att[b][:, cols] = r["att"]
    return (c_att, att), res


def kernel(x, y, Wq, Wk, Wv):
    out, _ = run_cores(x, y, Wq, Wk, Wv)
    return out
